# revision 3
# baseline (speedup 1.0000x reference)
"""Batched dynamic-weight depthwise cross-correlation on 8 trn2 NeuronCores.

out[b, y, x, c] = sum_{i,j} search[b, y+i, x+j, c] * template[b, i, j, c]
search: (128, 31, 31, 256) f32, template: (128, 7, 7, 256) f32 -> (128, 25, 25, 256) f32

Sharding: pure data parallel over batch (16 per core).

Per-core kernel (per unit = one (batch, channel-group-of-128)):
  1. DMA search/template in natural [spatial, c] layout, PE-transpose to
     [c, spatial]; ACT evacuates to bf16. DVE builds an fp8 hi/lo pair of
     planes s8hl = (fp8(s), fp8(s - fp8(s))) whose sum is s to ~bf16
     accuracy.
  2. 49 taps split between:
     - PE fp8 DoubleRow (N8 taps): stationary = diag(fp8(t)) duplicated in
       both K-halves, moving = both s8hl planes of the tap window. One
       matmul computes diag(t8) @ (s_hi + s_lo) at 0.5 cyc/row.
     - PE bf16 (rest of rows 0-5): diag(t) matmuls at 1 cyc/row.
     - DVE (row 6): scalar_tensor_tensor MAC chain on an f32 acc.
     Stationaries built on Pool/ACT/DVE via tensor_scalar_mul.
  3. PE psum partials evacuate (ACT) to bf16, DVE folds them into acc,
     5 f32r PE-transposes move acc to [spatial, c] psum, ACT converts,
     DMA out.
"""

import numpy as np

import concourse.bacc as bacc
import concourse.bass as bass
import concourse.tile as tile
from concourse import mybir
from concourse.ap import AP
from concourse.bass_utils import run_bass_kernel_spmd

K = 7
X = 31
O = 25  # X - K + 1
B = 128
C = 256
N_CORES = 8
BL = B // N_CORES  # 16 batches per core
CG = C // 128  # 2 channel groups
F32 = mybir.dt.float32
F32R = mybir.dt.float32r
BF16 = mybir.dt.bfloat16
FP8 = mybir.dt.float8e4

YSPLIT = 13  # pa rows (y 0..12), pb rows (y 13..24)
XW = O

# ---- tap assignment (tunable) ---------------------------------------------
N8 = 28  # taps on the fp8-DoubleRow path (rows 0-5; max 42)
ROWS05 = [(i, j) for i in range(6) for j in range(K)]
FP8_TAPS = ROWS05[:N8]
BF16_TAPS = ROWS05[N8:]
DVE_TAPS = [(6, j) for j in range(K)]

# stationary-build engine split (slots ordered: fp8 w8 builds, then bf16 diags)
W8_POOL = 12
W8_ACT = 12
DIAG_POOL = 8
DIAG_ACT = 2

# Pool buffer counts
PST_BUFS = 2
SNAT_BUFS = 2
DIAG_BUFS = 10
W8_BUFS = 10
ONAT_BUFS = 3


def _build_bass():
    nc = bacc.Bacc("TRN2", target_bir_lowering=False, debug=False)

    search = nc.dram_tensor("search", [BL, X, X, C], F32, kind="ExternalInput")
    template = nc.dram_tensor("template", [BL, K, K, C], F32, kind="ExternalInput")
    eye = nc.dram_tensor("eye", [128, 128], F32, kind="ExternalInput")
    out = nc.dram_tensor("out", [BL, O, O, C], F32, kind="ExternalOutput")

    s_flat = search.ap().rearrange("b y x c -> b (y x) c")  # [BL, 961, C]
    t_flat = template.ap().rearrange("b i j c -> b (i j) c")  # [BL, 49, C]
    o_flat = out.ap().rearrange("b y x c -> b (y x) c")  # [BL, 625, C]

    SP = X * X  # 961
    in_chunks = [(k * 128, min(128, SP - k * 128)) for k in range((SP + 127) // 128)]
    OP = O * O  # 625
    out_chunks = [(k * 128, min(128, OP - k * 128)) for k in range((OP + 127) // 128)]

    def w8_engine(slot):
        if slot < W8_POOL:
            return "pool"
        if slot < W8_POOL + W8_ACT:
            return "act"
        return "dve"

    def diag_engine(slot):
        if slot < DIAG_POOL:
            return "pool"
        if slot < DIAG_POOL + DIAG_ACT:
            return "act"
        return "dve"

    with tile.TileContext(nc) as tc:
        with (
            tc.tile_pool(name="singles", bufs=1) as singles,
            tc.tile_pool(name="p_snat", bufs=SNAT_BUFS) as p_snat,
            tc.tile_pool(name="p_st", bufs=PST_BUFS) as p_st,
            tc.tile_pool(name="p_s8", bufs=PST_BUFS) as p_s8,
            tc.tile_pool(name="p_tnat", bufs=2) as p_tnat,
            tc.tile_pool(name="p_tt", bufs=2) as p_tt,
            tc.tile_pool(name="p_diag", bufs=DIAG_BUFS) as p_diag,
            tc.tile_pool(name="p_w8", bufs=W8_BUFS) as p_w8,
            tc.tile_pool(name="p_acc", bufs=2) as p_acc,
            tc.tile_pool(name="p_acc2", bufs=2) as p_acc2,
            tc.tile_pool(name="p_onat", bufs=ONAT_BUFS) as p_onat,
            tc.tile_pool(name="ps_tin", bufs=1, space="PSUM") as ps_tin,
            tc.tile_pool(name="ps_acc", bufs=2, space="PSUM") as ps_acc,
            tc.tile_pool(name="ps_out", bufs=1, space="PSUM") as ps_out,
        ):
            eye_sb = singles.tile([128, 128], F32)
            nc.sync.dma_start(out=eye_sb[:], in_=eye.ap()[:, :])
            eye_r = singles.tile([128, 128], F32R)
            nc.scalar.copy(out=eye_r[:], in_=eye_sb[:])
            eye_b = singles.tile([128, 128], BF16)
            nc.scalar.copy(out=eye_b[:], in_=eye_sb[:])
            # eye doubled along a stride-0 middle dim for fused w8 builds
            eb = eye_b[:]
            eye_b2 = AP(eb.tensor, eb.offset, [list(eb.ap[0]), [0, 2], [1, 128]])

            def emit_output(st):
                b, c0, acc = st
                ot = ps_out.tile([128, 5, 128], F32R, tag="ot", name="ot")
                for ck, (r0, rows) in enumerate(out_chunks):
                    nc.tensor.matmul(
                        ot[:rows, ck, :],
                        acc[:, r0 : r0 + rows],
                        eye_r[:, :],
                        is_transpose=True,
                        start=True,
                        stop=True,
                    )
                o_nat = p_onat.tile([128, 5, 128], F32, tag="o_nat", name="o_nat")
                nc.scalar.copy(
                    out=o_nat[:].rearrange("p k c -> p (k c)")[:, :512],
                    in_=ot[:].rearrange("p k c -> p (k c)")[:, :512],
                )
                nc.scalar.copy(
                    out=o_nat[:113].rearrange("p k c -> p (k c)")[:, 512:640],
                    in_=ot[:113].rearrange("p k c -> p (k c)")[:, 512:640],
                )
                nc.sync.dma_start(
                    out=o_flat[b, 0:512, c0 : c0 + 128].rearrange(
                        "(k r) c -> r k c", k=4
                    ),
                    in_=o_nat[:, :4, :],
                )
                nc.sync.dma_start(
                    out=o_flat[b, 512:OP, c0 : c0 + 128].rearrange(
                        "(k r) c -> r k c", k=1
                    ),
                    in_=o_nat[:113, 4:5, :],
                )

            pending = None
            for b in range(BL):
                s_nat = p_snat.tile([128, 8, C], F32R)
                nc.sync.dma_start(
                    out=s_nat[:, :7, :],
                    in_=s_flat[b, 0:896, :].bitcast(F32R).rearrange(
                        "(k r) c -> r k c", k=7
                    ),
                )
                nc.sync.dma_start(
                    out=s_nat[:65, 7:8, :],
                    in_=s_flat[b, 896:SP, :].bitcast(F32R).rearrange(
                        "(k r) c -> r k c", k=1
                    ),
                )
                t_nat = p_tnat.tile([49, C], F32R)
                nc.sync.dma_start(out=t_nat[:], in_=t_flat[b, :, :].bitcast(F32R))

                for cg in range(CG):
                    c0 = cg * 128
                    with tc.high_priority(offset=52):
                        pt_t = ps_tin.tile([128, 1024], F32R, tag="ps_tin")
                        nc.tensor.transpose(
                            pt_t[:, 962 : 962 + 50],
                            t_nat[:, c0 : c0 + 128],
                            eye_r[:49, :50],
                        )
                        t_t = p_tt.tile([128, 49], F32)
                        nc.scalar.copy(out=t_t[:], in_=pt_t[:, 962 : 962 + 49])

                        for k, (r0, rows) in enumerate(in_chunks):
                            n_ev = rows + (rows % 2)
                            nc.tensor.transpose(
                                pt_t[:, r0 : r0 + n_ev],
                                s_nat[:rows, k, c0 : c0 + 128],
                                eye_r[:rows, :n_ev],
                            )
                        pt3 = pt_t[:, :SP].rearrange("p (y x) -> p y x", x=X)
                        s_bf = p_st.tile([128, X, 32], BF16)
                        nc.scalar.copy(out=s_bf[:, :, :X], in_=pt3[:, :, :])
                        # fp8 hi/lo planes: hi = fp8(s_bf), lo = fp8(s_bf - hi)
                        s8 = p_s8.tile([128, 2, X, 32], FP8)
                        nc.vector.tensor_copy(
                            out=s8[:, 0, :, :X], in_=s_bf[:, :, :X]
                        )
                        nc.vector.tensor_tensor(
                            out=s8[:, 1, :, :X],
                            in0=s_bf[:, :, :X],
                            in1=s8[:, 0, :, :X],
                            op=mybir.AluOpType.subtract,
                        )

                    # ---- stationaries
                    w8_tiles = []
                    for slot, (i, j) in enumerate(FP8_TAPS):
                        ij = i * K + j
                        w8 = p_w8.tile([128, 2, 128], FP8, tag="w8")
                        eng = w8_engine(slot)
                        if eng == "pool":
                            nc.gpsimd.tensor_scalar_mul(
                                out=w8[:], in0=eye_b2, scalar1=t_t[:, ij : ij + 1]
                            )
                        elif eng == "act":
                            nc.scalar.mul(
                                out=w8[:], in_=eye_b2, mul=t_t[:, ij : ij + 1]
                            )
                        else:
                            nc.vector.tensor_scalar_mul(
                                out=w8[:], in0=eye_b2, scalar1=t_t[:, ij : ij + 1]
                            )
                        w8_tiles.append(w8)
                    diag_tiles = []
                    for slot, (i, j) in enumerate(BF16_TAPS):
                        ij = i * K + j
                        diag = p_diag.tile([128, 128], BF16, tag="diag")
                        eng = diag_engine(slot)
                        if eng == "pool":
                            nc.gpsimd.tensor_scalar_mul(
                                out=diag[:], in0=eye_b[:], scalar1=t_t[:, ij : ij + 1]
                            )
                        elif eng == "act":
                            nc.scalar.mul(
                                out=diag[:], in_=eye_b[:], mul=t_t[:, ij : ij + 1]
                            )
                        else:
                            nc.vector.tensor_scalar_mul(
                                out=diag[:], in0=eye_b[:], scalar1=t_t[:, ij : ij + 1]
                            )
                        diag_tiles.append(diag)

                    # ---- PE taps accumulate into pa/pb psum.
                    pa = ps_acc.tile([128, YSPLIT, XW], F32, tag="pa")
                    pb = ps_acc.tile([128, O - YSPLIT, XW], F32, tag="pb")
                    n_mm = len(FP8_TAPS) + len(BF16_TAPS)
                    mi = 0
                    s8ap = s8[:]
                    pstr = list(s8ap.ap[0])
                    PLANE = X * 32
                    for ti, (i, j) in enumerate(FP8_TAPS):
                        first = mi == 0
                        last = mi == n_mm - 1
                        mi += 1
                        base_off = s8ap.offset + i * 32 + j
                        mv_a = AP(
                            s8ap.tensor,
                            base_off,
                            [pstr, [PLANE, 2], [32, YSPLIT], [1, XW]],
                        )
                        mv_b = AP(
                            s8ap.tensor,
                            base_off + YSPLIT * 32,
                            [pstr, [PLANE, 2], [32, O - YSPLIT], [1, XW]],
                        )
                        nc.tensor.matmul(
                            pa[:],
                            w8_tiles[ti][:],
                            mv_a,
                            start=first,
                            stop=last,
                            perf_mode=mybir.MatmulPerfMode.DoubleRow,
                        )
                        nc.tensor.matmul(
                            pb[:],
                            w8_tiles[ti][:],
                            mv_b,
                            start=first,
                            stop=last,
                            perf_mode=mybir.MatmulPerfMode.DoubleRow,
                        )
                    for ti, (i, j) in enumerate(BF16_TAPS):
                        first = mi == 0
                        last = mi == n_mm - 1
                        mi += 1
                        nc.tensor.matmul(
                            pa[:],
                            diag_tiles[ti][:],
                            s_bf[:, i : i + YSPLIT, j : j + XW],
                            start=first,
                            stop=last,
                        )
                        nc.tensor.matmul(
                            pb[:],
                            diag_tiles[ti][:],
                            s_bf[:, i + YSPLIT : i + O, j : j + XW],
                            start=first,
                            stop=last,
                        )

                    # ---- DVE taps: STT MAC chain on f32 acc.
                    acc = p_acc.tile([128, OP], F32R)
                    a3 = acc[:].rearrange("p (y x) -> p y x", x=O)
                    a3f = acc[:].bitcast(F32).rearrange("p (y x) -> p y x", x=O)
                    for n, (i, j) in enumerate(DVE_TAPS):
                        ij = i * K + j
                        win = s_bf[:, i : i + O, j : j + O]
                        if n == 0:
                            nc.vector.tensor_scalar_mul(
                                out=a3[:], in0=win, scalar1=t_t[:, ij : ij + 1]
                            )
                        else:
                            nc.vector.scalar_tensor_tensor(
                                out=a3[:],
                                in0=win,
                                scalar=t_t[:, ij : ij + 1],
                                in1=a3f[:],
                                op0=mybir.AluOpType.mult,
                                op1=mybir.AluOpType.add,
                            )

                    # ---- fold PE psum into acc: ACT evac to bf16, DVE add.
                    acc2 = p_acc2.tile([128, OP], BF16, name="acc2")
                    a23 = acc2[:].rearrange("p (y x) -> p y x", x=O)
                    nc.scalar.copy(out=a23[:, :YSPLIT, :], in_=pa[:])
                    nc.scalar.copy(out=a23[:, YSPLIT:, :], in_=pb[:])
                    nc.vector.tensor_tensor(
                        out=acc[:],
                        in0=acc[:].bitcast(F32),
                        in1=acc2[:],
                        op=mybir.AluOpType.add,
                    )

                    if pending is not None:
                        emit_output(pending)
                    pending = (b, c0, acc)
            emit_output(pending)
    nc.compile()
    return nc


_NC_CACHE = None


def _get_nc():
    global _NC_CACHE
    if _NC_CACHE is None:
        _NC_CACHE = _build_bass()
    return _NC_CACHE


def _run(search: np.ndarray, template: np.ndarray, **spmd_kwargs):
    nc = _get_nc()
    search = np.ascontiguousarray(np.asarray(search), dtype=np.float32)
    template = np.ascontiguousarray(np.asarray(template), dtype=np.float32)
    eye = np.eye(128, dtype=np.float32)
    in_maps = [
        {
            "search": search[c * BL : (c + 1) * BL],
            "template": template[c * BL : (c + 1) * BL],
            "eye": eye,
        }
        for c in range(N_CORES)
    ]
    res = run_bass_kernel_spmd(nc, in_maps, core_ids=list(range(N_CORES)), **spmd_kwargs)
    out = np.concatenate([r["out"] for r in res.results], axis=0)
    return out, res


def kernel(search: np.ndarray, template: np.ndarray) -> np.ndarray:
    out, _ = _run(search, template)
    return out


# revision 11
# speedup vs baseline: 1.2140x; 1.2140x over previous
"""Batched dynamic-weight depthwise cross-correlation on 8 trn2 NeuronCores.

out[b, y, x, c] = sum_{i,j} search[b, y+i, x+j, c] * template[b, i, j, c]
search: (128, 31, 31, 256) f32, template: (128, 7, 7, 256) f32 -> (128, 25, 25, 256) f32

Sharding: pure data parallel over batch (16 per core).

Per-core kernel (per unit = one (batch, channel-group-of-128)):
  1. DMA search/template in natural [spatial, c] layout, PE-transpose to
     [c, spatial]; ACT evacuates to bf16. DVE builds an fp8 hi/lo pair of
     planes s8hl = (fp8(s), fp8(s - fp8(s))) whose sum is s to ~bf16
     accuracy.
  2. 49 taps split between:
     - PE fp8 DoubleRow (N8 taps): stationary = diag(fp8(t)) duplicated in
       both K-halves, moving = both s8hl planes of the tap window. One
       matmul computes diag(t8) @ (s_hi + s_lo) at 0.5 cyc/row.
     - PE bf16 (rest of rows 0-5): diag(t) matmuls at 1 cyc/row.
     - DVE (row 6): scalar_tensor_tensor MAC chain on an f32 acc.
     Stationaries built on Pool/ACT/DVE via tensor_scalar_mul.
  3. PE psum partials evacuate (ACT) to bf16, DVE folds them into acc,
     5 f32r PE-transposes move acc to [spatial, c] psum, ACT converts,
     DMA out.
"""

import numpy as np

import concourse.bacc as bacc
import concourse.bass as bass
import concourse.tile as tile
from concourse import mybir
from concourse.ap import AP
from concourse.bass_utils import run_bass_kernel_spmd

K = 7
X = 31
O = 25  # X - K + 1
B = 128
C = 256
N_CORES = 8
BL = B // N_CORES  # 16 batches per core
CG = C // 128  # 2 channel groups
F32 = mybir.dt.float32
F32R = mybir.dt.float32r
BF16 = mybir.dt.bfloat16
FP8 = mybir.dt.float8e4

YSPLIT = 13  # pa rows (y 0..12), pb rows (y 13..24)
XW = O

# ---- tap assignment (tunable) ---------------------------------------------
N8 = 32  # taps on the fp8-DoubleRow path (rows 0-5; max 42)
ROWS05 = [(i, j) for i in range(6) for j in range(K)]
FP8_TAPS = ROWS05[:N8]
BF16_TAPS = ROWS05[N8:]
DVE_TAPS = [(6, j) for j in range(K)]

# stationary-build engine split (slots ordered: fp8 w8 builds, then bf16 diags)
W8_POOL = 20
W8_ACT = 3
DIAG_POOL = 7
DIAG_ACT = 3
TT_ENGINE = "act"  # t_t evacuation from psum (gpsimd cannot access PSUM)

# Pool buffer counts
PST_BUFS = 2
SNAT_BUFS = 2
DIAG_BUFS = 10
W8_BUFS = 10
ONAT_BUFS = 3


def _build_bass():
    nc = bacc.Bacc("TRN2", target_bir_lowering=False, debug=False)

    search = nc.dram_tensor("search", [BL, X, X, C], F32, kind="ExternalInput")
    template = nc.dram_tensor("template", [BL, K, K, C], F32, kind="ExternalInput")
    eye = nc.dram_tensor("eye", [128, 128], F32, kind="ExternalInput")
    out = nc.dram_tensor("out", [BL, O, O, C], F32, kind="ExternalOutput")

    s_flat = search.ap().rearrange("b y x c -> b (y x) c")  # [BL, 961, C]
    t_flat = template.ap().rearrange("b i j c -> b (i j) c")  # [BL, 49, C]
    o_flat = out.ap().rearrange("b y x c -> b (y x) c")  # [BL, 625, C]

    SP = X * X  # 961
    in_chunks = [(k * 128, min(128, SP - k * 128)) for k in range((SP + 127) // 128)]
    OP = O * O  # 625
    out_chunks = [(k * 128, min(128, OP - k * 128)) for k in range((OP + 127) // 128)]

    def w8_engine(slot):
        if slot < W8_POOL:
            return "pool"
        if slot < W8_POOL + W8_ACT:
            return "act"
        return "dve"

    def diag_engine(slot):
        if slot < DIAG_POOL:
            return "pool"
        if slot < DIAG_POOL + DIAG_ACT:
            return "act"
        return "dve"

    with tile.TileContext(nc) as tc:
        with (
            tc.tile_pool(name="singles", bufs=1) as singles,
            tc.tile_pool(name="p_snat", bufs=SNAT_BUFS) as p_snat,
            tc.tile_pool(name="p_st", bufs=PST_BUFS) as p_st,
            tc.tile_pool(name="p_s8", bufs=PST_BUFS) as p_s8,
            tc.tile_pool(name="p_tnat", bufs=2) as p_tnat,
            tc.tile_pool(name="p_tt", bufs=2) as p_tt,
            tc.tile_pool(name="p_diag", bufs=DIAG_BUFS) as p_diag,
            tc.tile_pool(name="p_w8", bufs=W8_BUFS) as p_w8,
            tc.tile_pool(name="p_acc", bufs=2) as p_acc,
            tc.tile_pool(name="p_onat", bufs=ONAT_BUFS) as p_onat,
            tc.tile_pool(name="ps_tin", bufs=1, space="PSUM") as ps_tin,
            tc.tile_pool(name="ps_acc", bufs=2, space="PSUM") as ps_acc,
            tc.tile_pool(name="ps_out", bufs=1, space="PSUM") as ps_out,
        ):
            eye_sb = singles.tile([128, 128], F32)
            nc.sync.dma_start(out=eye_sb[:], in_=eye.ap()[:, :])
            eye_r = singles.tile([128, 128], F32R)
            nc.scalar.copy(out=eye_r[:], in_=eye_sb[:])
            eye_b = singles.tile([128, 128], BF16)
            nc.scalar.copy(out=eye_b[:], in_=eye_sb[:])

            def emit_output(st):
                b, c0, acc = st
                ot = ps_out.tile([128, 5, 128], F32R, tag="ot", name="ot")
                for ck, (r0, rows) in enumerate(out_chunks):
                    nc.tensor.matmul(
                        ot[:rows, ck, :],
                        acc[:, r0 : r0 + rows],
                        eye_r[:, :],
                        is_transpose=True,
                        start=True,
                        stop=True,
                    )
                o_nat = p_onat.tile([128, 5, 128], F32, tag="o_nat", name="o_nat")
                nc.scalar.copy(
                    out=o_nat[:].rearrange("p k c -> p (k c)")[:, :512],
                    in_=ot[:].rearrange("p k c -> p (k c)")[:, :512],
                )
                nc.scalar.copy(
                    out=o_nat[:113].rearrange("p k c -> p (k c)")[:, 512:640],
                    in_=ot[:113].rearrange("p k c -> p (k c)")[:, 512:640],
                )
                nc.sync.dma_start(
                    out=o_flat[b, 0:512, c0 : c0 + 128].rearrange(
                        "(k r) c -> r k c", k=4
                    ),
                    in_=o_nat[:, :4, :],
                )
                nc.sync.dma_start(
                    out=o_flat[b, 512:OP, c0 : c0 + 128].rearrange(
                        "(k r) c -> r k c", k=1
                    ),
                    in_=o_nat[:113, 4:5, :],
                )

            pending = None
            for b in range(BL):
                s_nat = p_snat.tile([128, 8, C], F32R)
                nc.sync.dma_start(
                    out=s_nat[:, :7, :],
                    in_=s_flat[b, 0:896, :].bitcast(F32R).rearrange(
                        "(k r) c -> r k c", k=7
                    ),
                )
                nc.sync.dma_start(
                    out=s_nat[:65, 7:8, :],
                    in_=s_flat[b, 896:SP, :].bitcast(F32R).rearrange(
                        "(k r) c -> r k c", k=1
                    ),
                )
                t_nat = p_tnat.tile([49, C], F32R)
                nc.sync.dma_start(out=t_nat[:], in_=t_flat[b, :, :].bitcast(F32R))

                for cg in range(CG):
                    c0 = cg * 128
                    with tc.high_priority(offset=52):
                        pt_t = ps_tin.tile([128, 1024], F32R, tag="ps_tin")
                        nc.tensor.transpose(
                            pt_t[:, 962 : 962 + 50],
                            t_nat[:, c0 : c0 + 128],
                            eye_r[:49, :50],
                        )
                        t_t = p_tt.tile([128, 49], F32)
                        if TT_ENGINE == "pool":
                            nc.gpsimd.tensor_copy(
                                out=t_t[:], in_=pt_t[:, 962 : 962 + 49]
                            )
                        else:
                            nc.scalar.copy(out=t_t[:], in_=pt_t[:, 962 : 962 + 49])

                        for k, (r0, rows) in enumerate(in_chunks):
                            n_ev = rows + (rows % 2)
                            nc.tensor.transpose(
                                pt_t[:, r0 : r0 + n_ev],
                                s_nat[:rows, k, c0 : c0 + 128],
                                eye_r[:rows, :n_ev],
                            )
                        pt3 = pt_t[:, :SP].rearrange("p (y x) -> p y x", x=X)
                        s_bf = p_st.tile([128, X, 32], BF16)
                        nc.scalar.copy(out=s_bf[:, :, :X], in_=pt3[:, :, :])
                        # fp8 hi/lo planes: hi = fp8(s_bf), lo = fp8(s_bf - hi)
                        s8 = p_s8.tile([128, 2, X, 32], FP8)
                        nc.scalar.copy(out=s8[:, 0, :, :X], in_=s_bf[:, :, :X])
                        nc.vector.tensor_tensor(
                            out=s8[:, 1, :, :X],
                            in0=s_bf[:, :, :X],
                            in1=s8[:, 0, :, :X],
                            op=mybir.AluOpType.subtract,
                        )

                    # ---- stationaries (single-plane fp8 diags; the DoubleRow
                    # matmul reads them twice via a stride-0 Ko dim)
                    w8_tiles = []
                    for slot, (i, j) in enumerate(FP8_TAPS):
                        ij = i * K + j
                        w8 = p_w8.tile([128, 128], FP8, tag="w8")
                        eng = w8_engine(slot)
                        if eng == "pool":
                            nc.gpsimd.tensor_scalar_mul(
                                out=w8[:], in0=eye_b[:], scalar1=t_t[:, ij : ij + 1]
                            )
                        elif eng == "act":
                            nc.scalar.mul(
                                out=w8[:], in_=eye_b[:], mul=t_t[:, ij : ij + 1]
                            )
                        else:
                            nc.vector.tensor_scalar_mul(
                                out=w8[:], in0=eye_b[:], scalar1=t_t[:, ij : ij + 1]
                            )
                        w8_tiles.append(w8)
                    diag_tiles = []
                    for slot, (i, j) in enumerate(BF16_TAPS):
                        ij = i * K + j
                        diag = p_diag.tile([128, 128], BF16, tag="diag")
                        eng = diag_engine(slot)
                        if eng == "pool":
                            nc.gpsimd.tensor_scalar_mul(
                                out=diag[:], in0=eye_b[:], scalar1=t_t[:, ij : ij + 1]
                            )
                        elif eng == "act":
                            nc.scalar.mul(
                                out=diag[:], in_=eye_b[:], mul=t_t[:, ij : ij + 1]
                            )
                        else:
                            nc.vector.tensor_scalar_mul(
                                out=diag[:], in0=eye_b[:], scalar1=t_t[:, ij : ij + 1]
                            )
                        diag_tiles.append(diag)

                    # ---- PE taps accumulate into pa/pb psum.
                    pa = ps_acc.tile([128, YSPLIT, XW], F32, tag="pa")
                    pb = ps_acc.tile([128, O - YSPLIT, XW], F32, tag="pb")
                    n_mm = len(FP8_TAPS) + len(BF16_TAPS)
                    mi = 0
                    s8ap = s8[:]
                    pstr = list(s8ap.ap[0])
                    PLANE = X * 32
                    for ti, (i, j) in enumerate(FP8_TAPS):
                        first = mi == 0
                        last = mi == n_mm - 1
                        mi += 1
                        base_off = s8ap.offset + i * 32 + j
                        mv_a = AP(
                            s8ap.tensor,
                            base_off,
                            [pstr, [PLANE, 2], [32, YSPLIT], [1, XW]],
                        )
                        mv_b = AP(
                            s8ap.tensor,
                            base_off + YSPLIT * 32,
                            [pstr, [PLANE, 2], [32, O - YSPLIT], [1, XW]],
                        )
                        w8ap = w8_tiles[ti][:]
                        w8d = AP(
                            w8ap.tensor,
                            w8ap.offset,
                            [list(w8ap.ap[0]), [0, 2], [1, 128]],
                        )
                        nc.tensor.matmul(
                            pa[:],
                            w8d,
                            mv_a,
                            start=first,
                            stop=last,
                            perf_mode=mybir.MatmulPerfMode.DoubleRow,
                        )
                        nc.tensor.matmul(
                            pb[:],
                            w8d,
                            mv_b,
                            start=first,
                            stop=last,
                            perf_mode=mybir.MatmulPerfMode.DoubleRow,
                        )
                    for ti, (i, j) in enumerate(BF16_TAPS):
                        first = mi == 0
                        last = mi == n_mm - 1
                        mi += 1
                        nc.tensor.matmul(
                            pa[:],
                            diag_tiles[ti][:],
                            s_bf[:, i : i + YSPLIT, j : j + XW],
                            start=first,
                            stop=last,
                        )
                        nc.tensor.matmul(
                            pb[:],
                            diag_tiles[ti][:],
                            s_bf[:, i + YSPLIT : i + O, j : j + XW],
                            start=first,
                            stop=last,
                        )

                    # ---- DVE taps: STT MAC chain on f32 acc.
                    acc = p_acc.tile([128, OP], F32R)
                    a3 = acc[:].rearrange("p (y x) -> p y x", x=O)
                    a3f = acc[:].bitcast(F32).rearrange("p (y x) -> p y x", x=O)
                    for n, (i, j) in enumerate(DVE_TAPS):
                        ij = i * K + j
                        win = s_bf[:, i : i + O, j : j + O]
                        if n == 0:
                            nc.vector.tensor_scalar_mul(
                                out=a3[:], in0=win, scalar1=t_t[:, ij : ij + 1]
                            )
                        else:
                            nc.vector.scalar_tensor_tensor(
                                out=a3[:],
                                in0=win,
                                scalar=t_t[:, ij : ij + 1],
                                in1=a3f[:],
                                op0=mybir.AluOpType.mult,
                                op1=mybir.AluOpType.add,
                            )

                    # ---- fold PE psum into acc directly on DVE.
                    nc.vector.tensor_tensor(
                        out=a3[:, :YSPLIT, :],
                        in0=a3f[:, :YSPLIT, :],
                        in1=pa[:],
                        op=mybir.AluOpType.add,
                    )
                    nc.vector.tensor_tensor(
                        out=a3[:, YSPLIT:, :],
                        in0=a3f[:, YSPLIT:, :],
                        in1=pb[:],
                        op=mybir.AluOpType.add,
                    )

                    if pending is not None:
                        emit_output(pending)
                    pending = (b, c0, acc)
            emit_output(pending)
    nc.compile()
    return nc


_NC_CACHE = None


def _get_nc():
    global _NC_CACHE
    if _NC_CACHE is None:
        _NC_CACHE = _build_bass()
    return _NC_CACHE


def _run(search: np.ndarray, template: np.ndarray, **spmd_kwargs):
    nc = _get_nc()
    search = np.ascontiguousarray(np.asarray(search), dtype=np.float32)
    template = np.ascontiguousarray(np.asarray(template), dtype=np.float32)
    eye = np.eye(128, dtype=np.float32)
    in_maps = [
        {
            "search": search[c * BL : (c + 1) * BL],
            "template": template[c * BL : (c + 1) * BL],
            "eye": eye,
        }
        for c in range(N_CORES)
    ]
    res = run_bass_kernel_spmd(nc, in_maps, core_ids=list(range(N_CORES)), **spmd_kwargs)
    out = np.concatenate([r["out"] for r in res.results], axis=0)
    return out, res


def kernel(search: np.ndarray, template: np.ndarray) -> np.ndarray:
    out, _ = _run(search, template)
    return out


# revision 22
# speedup vs baseline: 1.4088x; 1.1605x over previous
"""Batched dynamic-weight depthwise cross-correlation on 8 trn2 NeuronCores.

out[b, y, x, c] = sum_{i,j} search[b, y+i, x+j, c] * template[b, i, j, c]
search: (128, 31, 31, 256) f32, template: (128, 7, 7, 256) f32 -> (128, 25, 25, 256) f32

Sharding: pure data parallel over batch (16 per core).

Per-core kernel (per unit = one (batch, channel-group-of-128)):
  1. DMA search/template in natural [spatial, c] layout, PE-transpose to
     [c, spatial]; ACT evacuates to bf16. DVE builds an fp8 hi/lo pair of
     planes s8hl = (fp8(s), fp8(s - fp8(s))) whose sum is s to ~bf16
     accuracy.
  2. 49 taps split between:
     - PE fp8 DoubleRow (N8 taps): stationary = diag(fp8(t)) duplicated in
       both K-halves, moving = both s8hl planes of the tap window. One
       matmul computes diag(t8) @ (s_hi + s_lo) at 0.5 cyc/row.
     - PE bf16 (rest of rows 0-5): diag(t) matmuls at 1 cyc/row.
     - DVE (row 6): scalar_tensor_tensor MAC chain on an f32 acc.
     Stationaries built on Pool/ACT/DVE via tensor_scalar_mul.
  3. PE psum partials evacuate (ACT) to bf16, DVE folds them into acc,
     5 f32r PE-transposes move acc to [spatial, c] psum, ACT converts,
     DMA out.
"""

import numpy as np

import concourse.bacc as bacc
import concourse.bass as bass
import concourse.tile as tile
from concourse import mybir
from concourse.ap import AP
from concourse.bass_utils import run_bass_kernel_spmd

K = 7
X = 31
O = 25  # X - K + 1
B = 128
C = 256
N_CORES = 8
BL = B // N_CORES  # 16 batches per core
CG = C // 128  # 2 channel groups
F32 = mybir.dt.float32
F32R = mybir.dt.float32r
BF16 = mybir.dt.bfloat16
FP8 = mybir.dt.float8e4

YSPLIT = 13  # pa rows (y 0..12), pb rows (y 13..24)
XW = O

# ---- tap assignment (tunable) ---------------------------------------------
N8 = 40  # taps on the fp8-DoubleRow path (rows 0-5; max 42)
ROWS05 = [(i, j) for i in range(6) for j in range(K)]
FP8_TAPS = ROWS05[:N8]
BF16_TAPS = ROWS05[N8:] + [(6, 6)]
DVE_TAPS = [(6, j) for j in range(K - 1)]

# stationary-build engine split (slots ordered: fp8 w8 builds, then bf16 diags)
W8_POOL = 25
W8_ACT = 12
DIAG_POOL = 0
DIAG_ACT = 0
TT_ENGINE = "act"  # t_t evacuation from psum (gpsimd cannot access PSUM)

# Pool buffer counts
PST_BUFS = 2
SNAT_BUFS = 3
DIAG_BUFS = 24
W8_BUFS = 48
ONAT_BUFS = 3


def _build_bass():
    nc = bacc.Bacc("TRN2", target_bir_lowering=False, debug=False)

    search = nc.dram_tensor("search", [BL, X, X, C], F32, kind="ExternalInput")
    template = nc.dram_tensor("template", [BL, K, K, C], F32, kind="ExternalInput")
    eye = nc.dram_tensor("eye", [128, 128], F32, kind="ExternalInput")
    out = nc.dram_tensor("out", [BL, O, O, C], F32, kind="ExternalOutput")

    s_flat = search.ap().rearrange("b y x c -> b (y x) c")  # [BL, 961, C]
    t_flat = template.ap().rearrange("b i j c -> b (i j) c")  # [BL, 49, C]
    o_flat = out.ap().rearrange("b y x c -> b (y x) c")  # [BL, 625, C]

    SP = X * X  # 961
    in_chunks = [(k * 128, min(128, SP - k * 128)) for k in range((SP + 127) // 128)]
    OP = O * O  # 625
    out_chunks = [(k * 128, min(128, OP - k * 128)) for k in range((OP + 127) // 128)]

    def w8_engine(slot):
        if slot < W8_POOL:
            return "pool"
        if slot < W8_POOL + W8_ACT:
            return "act"
        return "dve"

    def diag_engine(slot):
        if slot < DIAG_POOL:
            return "pool"
        if slot < DIAG_POOL + DIAG_ACT:
            return "act"
        return "dve"

    with tile.TileContext(nc) as tc:
        with (
            tc.tile_pool(name="singles", bufs=1) as singles,
            tc.tile_pool(name="p_snat", bufs=SNAT_BUFS) as p_snat,
            tc.tile_pool(name="p_st", bufs=PST_BUFS) as p_st,
            tc.tile_pool(name="p_s8", bufs=PST_BUFS) as p_s8,
            tc.tile_pool(name="p_tnat", bufs=2) as p_tnat,
            tc.tile_pool(name="p_tt", bufs=4) as p_tt,
            tc.tile_pool(name="p_diag", bufs=DIAG_BUFS) as p_diag,
            tc.tile_pool(name="p_w8", bufs=W8_BUFS) as p_w8,
            tc.tile_pool(name="p_acc", bufs=2) as p_acc,
            tc.tile_pool(name="p_onat", bufs=ONAT_BUFS) as p_onat,
            tc.tile_pool(name="ps_tin", bufs=1, space="PSUM") as ps_tin,
            tc.tile_pool(name="ps_acc", bufs=2, space="PSUM") as ps_acc,
            tc.tile_pool(name="ps_out", bufs=1, space="PSUM") as ps_out,
        ):
            eye_sb = singles.tile([128, 128], F32)
            nc.sync.dma_start(out=eye_sb[:], in_=eye.ap()[:, :])
            eye_r = singles.tile([128, 128], F32R)
            nc.scalar.copy(out=eye_r[:], in_=eye_sb[:])
            eye_b = singles.tile([128, 128], BF16)
            nc.scalar.copy(out=eye_b[:], in_=eye_sb[:])

            def emit_output(st):
                b, c0, acc = st
                ot = ps_out.tile([128, 5, 128], F32R, tag="ot", name="ot")
                for ck, (r0, rows) in enumerate(out_chunks):
                    nc.tensor.matmul(
                        ot[:rows, ck, :],
                        acc[:, r0 : r0 + rows],
                        eye_r[:, :],
                        is_transpose=True,
                        start=True,
                        stop=True,
                    )
                o_nat = p_onat.tile([128, 5, 128], F32, tag="o_nat", name="o_nat")
                nc.scalar.copy(
                    out=o_nat[:].rearrange("p k c -> p (k c)")[:, :512],
                    in_=ot[:].rearrange("p k c -> p (k c)")[:, :512],
                )
                nc.scalar.copy(
                    out=o_nat[:113].rearrange("p k c -> p (k c)")[:, 512:640],
                    in_=ot[:113].rearrange("p k c -> p (k c)")[:, 512:640],
                )
                nc.sync.dma_start(
                    out=o_flat[b, 0:512, c0 : c0 + 128].rearrange(
                        "(k r) c -> r k c", k=4
                    ),
                    in_=o_nat[:, :4, :],
                )
                nc.sync.dma_start(
                    out=o_flat[b, 512:OP, c0 : c0 + 128].rearrange(
                        "(k r) c -> r k c", k=1
                    ),
                    in_=o_nat[:113, 4:5, :],
                )

            pending = None
            for b in range(BL):
                s_nat = p_snat.tile([128, 8, C], F32R)
                nc.sync.dma_start(
                    out=s_nat[:, :7, :],
                    in_=s_flat[b, 0:896, :].bitcast(F32R).rearrange(
                        "(k r) c -> r k c", k=7
                    ),
                )
                nc.sync.dma_start(
                    out=s_nat[:65, 7:8, :],
                    in_=s_flat[b, 896:SP, :].bitcast(F32R).rearrange(
                        "(k r) c -> r k c", k=1
                    ),
                )
                t_nat = p_tnat.tile([49, C], F32R)
                nc.sync.dma_start(out=t_nat[:], in_=t_flat[b, :, :].bitcast(F32R))

                for cg in range(CG):
                    c0 = cg * 128
                    pa = ps_acc.tile([128, YSPLIT, XW], F32, tag="pa")
                    pb = ps_acc.tile([128, O - YSPLIT, XW], F32, tag="pb")
                    with tc.high_priority(offset=52):
                        paf = pa[:].rearrange("p y x -> p (y x)").bitcast(F32R)
                        nc.tensor.transpose(
                            paf[:, 0:50],
                            t_nat[:, c0 : c0 + 128],
                            eye_r[:49, :50],
                        )
                        t_t = p_tt.tile([128, 49], F32)
                        nc.scalar.copy(out=t_t[:], in_=paf[:, 0:49].bitcast(F32))
                        pt_t = ps_tin.tile([128, 1024], F32R, tag="ps_tin")

                        for k, (r0, rows) in enumerate(in_chunks):
                            n_ev = rows + (rows % 2)
                            nc.tensor.transpose(
                                pt_t[:, r0 : r0 + n_ev],
                                s_nat[:rows, k, c0 : c0 + 128],
                                eye_r[:rows, :n_ev],
                            )
                        pt3 = pt_t[:, :SP].rearrange("p (y x) -> p y x", x=X)
                        s_bf = p_st.tile([128, X, 32], BF16)
                        nc.scalar.copy(out=s_bf[:, :, :X], in_=pt3[:, :, :])
                        # fp8 hi/lo planes: hi = fp8(s_bf), lo = fp8(s_bf - hi)
                        s8 = p_s8.tile([128, 2, X, 32], FP8)
                        nc.scalar.copy(out=s8[:, 0, :, :X], in_=s_bf[:, :, :X])
                        nc.vector.tensor_tensor(
                            out=s8[:, 1, :, :X],
                            in0=s_bf[:, :, :X],
                            in1=s8[:, 0, :, :X],
                            op=mybir.AluOpType.subtract,
                        )

                    # ---- stationaries (single-plane fp8 diags; the DoubleRow
                    # matmul reads them twice via a stride-0 Ko dim)
                    w8_tiles = []
                    for slot, (i, j) in enumerate(FP8_TAPS):
                        ij = i * K + j
                        w8 = p_w8.tile([128, 128], FP8, tag="w8")
                        eng = w8_engine(slot)
                        if eng == "pool":
                            nc.gpsimd.tensor_scalar_mul(
                                out=w8[:], in0=eye_b[:], scalar1=t_t[:, ij : ij + 1]
                            )
                        elif eng == "act":
                            nc.scalar.mul(
                                out=w8[:], in_=eye_b[:], mul=t_t[:, ij : ij + 1]
                            )
                        else:
                            nc.vector.tensor_scalar_mul(
                                out=w8[:], in0=eye_b[:], scalar1=t_t[:, ij : ij + 1]
                            )
                        w8_tiles.append(w8)
                    diag_tiles = []
                    for slot, (i, j) in enumerate(BF16_TAPS):
                        ij = i * K + j
                        diag = p_diag.tile([128, 128], BF16, tag="diag")
                        eng = diag_engine(slot)
                        if eng == "pool":
                            nc.gpsimd.tensor_scalar_mul(
                                out=diag[:], in0=eye_b[:], scalar1=t_t[:, ij : ij + 1]
                            )
                        elif eng == "act":
                            nc.scalar.mul(
                                out=diag[:], in_=eye_b[:], mul=t_t[:, ij : ij + 1]
                            )
                        else:
                            nc.vector.tensor_scalar_mul(
                                out=diag[:], in0=eye_b[:], scalar1=t_t[:, ij : ij + 1]
                            )
                        diag_tiles.append(diag)

                    # ---- PE taps accumulate into pa/pb psum.
                    n_mm = len(FP8_TAPS) + len(BF16_TAPS)
                    mi = 0
                    s8ap = s8[:]
                    pstr = list(s8ap.ap[0])
                    PLANE = X * 32
                    for ti, (i, j) in enumerate(FP8_TAPS):
                        first = mi == 0
                        last = mi == n_mm - 1
                        mi += 1
                        base_off = s8ap.offset + i * 32 + j
                        mv_a = AP(
                            s8ap.tensor,
                            base_off,
                            [pstr, [PLANE, 2], [32, YSPLIT], [1, XW]],
                        )
                        mv_b = AP(
                            s8ap.tensor,
                            base_off + YSPLIT * 32,
                            [pstr, [PLANE, 2], [32, O - YSPLIT], [1, XW]],
                        )
                        w8ap = w8_tiles[ti][:]
                        w8d = AP(
                            w8ap.tensor,
                            w8ap.offset,
                            [list(w8ap.ap[0]), [0, 2], [1, 128]],
                        )
                        nc.tensor.matmul(
                            pa[:],
                            w8d,
                            mv_a,
                            start=first,
                            stop=last,
                            perf_mode=mybir.MatmulPerfMode.DoubleRow,
                        )
                        nc.tensor.matmul(
                            pb[:],
                            w8d,
                            mv_b,
                            start=first,
                            stop=last,
                            perf_mode=mybir.MatmulPerfMode.DoubleRow,
                        )
                    for ti, (i, j) in enumerate(BF16_TAPS):
                        first = mi == 0
                        last = mi == n_mm - 1
                        mi += 1
                        nc.tensor.matmul(
                            pa[:],
                            diag_tiles[ti][:],
                            s_bf[:, i : i + YSPLIT, j : j + XW],
                            start=first,
                            stop=last,
                        )
                        nc.tensor.matmul(
                            pb[:],
                            diag_tiles[ti][:],
                            s_bf[:, i + YSPLIT : i + O, j : j + XW],
                            start=first,
                            stop=last,
                        )

                    # ---- DVE taps: STT MAC chain on f32 acc.
                    acc = p_acc.tile([128, OP], F32R)
                    a3 = acc[:].rearrange("p (y x) -> p y x", x=O)
                    a3f = acc[:].bitcast(F32).rearrange("p (y x) -> p y x", x=O)
                    for n, (i, j) in enumerate(DVE_TAPS):
                        ij = i * K + j
                        win = s_bf[:, i : i + O, j : j + O]
                        if n == 0:
                            nc.vector.tensor_scalar_mul(
                                out=a3[:], in0=win, scalar1=t_t[:, ij : ij + 1]
                            )
                        else:
                            nc.vector.scalar_tensor_tensor(
                                out=a3[:],
                                in0=win,
                                scalar=t_t[:, ij : ij + 1],
                                in1=a3f[:],
                                op0=mybir.AluOpType.mult,
                                op1=mybir.AluOpType.add,
                            )
                    # fold PE psum into acc; prioritized so pa/pb free fast.
                    with tc.high_priority(offset=40):
                        nc.vector.tensor_tensor(
                            out=a3[:, :YSPLIT, :],
                            in0=a3f[:, :YSPLIT, :],
                            in1=pa[:],
                            op=mybir.AluOpType.add,
                        )
                        nc.vector.tensor_tensor(
                            out=a3[:, YSPLIT:, :],
                            in0=a3f[:, YSPLIT:, :],
                            in1=pb[:],
                            op=mybir.AluOpType.add,
                        )

                    if pending is not None:
                        emit_output(pending)
                    pending = (b, c0, acc)
            emit_output(pending)
    nc.compile()
    return nc


_NC_CACHE = None


def _get_nc():
    global _NC_CACHE
    if _NC_CACHE is None:
        _NC_CACHE = _build_bass()
    return _NC_CACHE


def _run(search: np.ndarray, template: np.ndarray, **spmd_kwargs):
    nc = _get_nc()
    search = np.ascontiguousarray(np.asarray(search), dtype=np.float32)
    template = np.ascontiguousarray(np.asarray(template), dtype=np.float32)
    eye = np.eye(128, dtype=np.float32)
    in_maps = [
        {
            "search": search[c * BL : (c + 1) * BL],
            "template": template[c * BL : (c + 1) * BL],
            "eye": eye,
        }
        for c in range(N_CORES)
    ]
    res = run_bass_kernel_spmd(nc, in_maps, core_ids=list(range(N_CORES)), **spmd_kwargs)
    out = np.concatenate([r["out"] for r in res.results], axis=0)
    return out, res


def kernel(search: np.ndarray, template: np.ndarray) -> np.ndarray:
    out, _ = _run(search, template)
    return out
